# revision 27
# baseline (speedup 1.0000x reference)
"""Trainium2 Bass kernel for additive (Bahdanau-style) attention scoring.

Computes, for hidden [B,H], encoder_outputs [B,S,H], W_attn [2H,H], b_attn [H], v [H]:
    energy    = tanh(hidden @ W1 + enc @ W2 + b_attn)   (per (b,s) row)
    attention = softmax_S(energy @ v)                   -> [B, S]

Sharding: data-parallel over batch across 8 NeuronCores (2 batches/core);
weights replicated.  Per-core compute is a 4096x1024x1024 GEMM + tanh +
v-dot + softmax, laid out as zT tiles [k=128 partitions, r free] so the
tanh bias is a per-partition AP on the scalar engine.  enc is cast
f32->fp16 during the HBM load (SWDGE) and transposed on-chip with PE
identity-matmul transposes whose PSUM results DVE-copies back to SBUF.

Improvements over the original baseline (224.5us -> ~194us on the same
device; the prologue is HBM-bound at ~285 GB/s effective, so DMA issue
order is everything):
- W1/W2 loaded with 2 big DMAs each instead of 8 small ones (each
  DMA_DIRECT2D costs ~1.2us of serial gpsimd queue time), ordered
  w2a -> tiny operands -> w2b -> w1a -> nat01 -> w1b so each consumer
  (first GEMM half, GEMM kc4-7, cbias via the psz-recycle slack,
  block-1 transposes) is fed just in time.
- nat pool bufs=8: every enc block's DMA issues up-front; no late
  mid-stream nat waits (was ~0.5us/block + 3.5us early).
- cbiasT computed directly as [k, b] (W1 chunk stationary, hidT
  moving), removing the extra PE transpose + second identity.
- Per-block exp+accum on the logits slice runs hidden under the next
  block's GEMM; the critical tail is only total/recip/scale/DMA.
- psz pool at 4 bufs (psep/psatt trimmed) for deeper GEMM/tanh overlap.

Measured dead ends (do not revisit): fp8 GEMM fails the 2e-2 gate
(8e-2); InstMatmult.ldweights=False is ignored by codegen (no
stationary reuse; ~46ns/matmul LDWEIGHTS exposure is a floor); DVE
tensor_scalar / scalar_tensor_tensor are ~10-40x slower than COPY;
GpSimd CROSS_LANE_REDUCE is ~77us per [128,512] tile; a [128,16]
partition-parallel softmax makes the output DMA a 4-byte scatter
(2048 descriptors, ~11.5us); HWDGE rings (sync/scalar only) do not
add bandwidth over the shared HBM limit.
"""

import sys
import types

import numpy as np

B, S, H = 16, 2048, 1024
N_CORES = 8
B_LOC = B // N_CORES  # 2 batches per core
HC = H // 128         # 8 contraction chunks
KC = H // 128         # 8 output-feature chunks
RB = 512              # rows (s positions) per block
NRB = S // RB         # 4 r-blocks per batch
NCH = S // 128        # 16 logit chunks of 128 per batch


def _ensure_axon_hooks():
    """Register the NTFF profile hook if the image's antenv lacks it."""
    try:
        import antenv.axon_hooks  # noqa: F401
        return
    except ImportError:
        pass
    try:
        import antenv
        from trn_agent_boot.trn_boot import _ntff_profile_via_ctypes
    except ImportError:
        return
    mod = types.ModuleType("antenv.axon_hooks")
    _hook = [None]
    mod.set_axon_ntff_profile_hook = lambda h: _hook.__setitem__(0, h)
    mod.get_axon_ntff_profile_hook = lambda: _hook[0]
    antenv.axon_hooks = mod
    sys.modules["antenv.axon_hooks"] = mod
    try:
        hook = _ntff_profile_via_ctypes("/opt/axon/libaxon_pjrt.so")
        mod.set_axon_ntff_profile_hook(hook)
    except Exception:
        pass


_ensure_axon_hooks()

import concourse.bass as bass  # noqa: E402,F401
import concourse.mybir as mybir  # noqa: E402
import concourse.tile as tile  # noqa: E402
from concourse import bacc  # noqa: E402
from concourse.bass_utils import run_bass_kernel_spmd  # noqa: E402
from concourse.masks import make_identity  # noqa: E402
from concourse.tile_rust import add_dep_helper  # noqa: E402

f32 = mybir.dt.float32
f16 = mybir.dt.float16
AF = mybir.ActivationFunctionType
ALU = mybir.AluOpType


def build_kernel():
    nc = bacc.Bacc("TRN2", target_bir_lowering=False, debug=False,
                   num_devices=N_CORES)

    enc = nc.dram_tensor("enc", [B_LOC, S, H], f32, kind="ExternalInput")
    hid = nc.dram_tensor("hid", [B_LOC, H], f32, kind="ExternalInput")
    w_attn = nc.dram_tensor("w_attn", [2 * H, H], f32, kind="ExternalInput")
    b_attn = nc.dram_tensor("b_attn", [H], f32, kind="ExternalInput")
    v = nc.dram_tensor("v", [H], f32, kind="ExternalInput")
    out = nc.dram_tensor("out", [B_LOC, S], f32, kind="ExternalOutput")

    with tile.TileContext(nc) as tc, \
         tc.tile_pool(name="weights", bufs=1) as wpool, \
         tc.tile_pool(name="consts", bufs=1) as cpool, \
         tc.tile_pool(name="nat", bufs=8) as natpool, \
         tc.tile_pool(name="encT", bufs=16) as tpool, \
         tc.tile_pool(name="energy", bufs=9) as epool, \
         tc.tile_pool(name="sm", bufs=1) as smpool, \
         tc.tile_pool(name="psz", bufs=4, space="PSUM") as pszpool, \
         tc.tile_pool(name="pst", bufs=2, space="PSUM") as pstpool, \
         tc.tile_pool(name="psatt", bufs=1, space="PSUM") as psattpool, \
         tc.tile_pool(name="pscb", bufs=1, space="PSUM") as pscbpool:

        # --- SWDGE FIFO ---------------------------------------------------
        # First enc block first (its transposes gate the whole pipeline),
        # identity for the PE transposes next, then big weight chunks.
        # Fewer + bigger DMAs: each DMA_DIRECT2D costs ~1.2us of gpsimd
        # queue time, and the whole prologue is HBM-bandwidth-bound
        # (~285 GB/s effective), so issue order = landing order.
        nat = {}

        def load_nat(b, rb, split=1):
            t = natpool.tile([128, (RB // 128) * H], f16, tag="nat")
            r0 = rb * RB
            step = (RB // 128) // split
            for s in range(split):
                nc.gpsimd.dma_start(
                    t[:].rearrange("p (j h) -> p j h", h=H)[
                        :, s * step:(s + 1) * step, :],
                    enc[b, r0 + s * step * 128:r0 + (s + 1) * step * 128, :]
                    .rearrange("(j p) h -> p j h", p=128))
            nat[(b, rb)] = t

        load_nat(0, 0, split=2)

        ident = cpool.tile([128, 128], f16, tag="ident")
        make_identity(nc, ident[:])

        # PE clock warm-up: the PE p-state ramps 0.65 -> 1.2 -> 2.4 GHz
        # only after ~3us of continuous work, and every DMA-bound stall
        # resets it.  Dependency-free identity self-transposes fill the
        # known HBM-bound stall windows (sized below the minimum observed
        # stall) so real work afterwards issues at full clock.
        def pe_warm(n):
            for i in range(0, n, 4):
                wt = pstpool.tile([128, RB], f16, tag="pst")
                for j in range(min(4, n - i)):
                    nc.tensor.transpose(
                        wt[:, j * 128:(j + 1) * 128], ident[:], ident[:])

        pe_warm(44)  # dead window before nat00 lands (~8us)

        # W2/W1 as two [128, (hc k)] half-k tiles each: slice for (kc, hc)
        # is [:, hc*512 + (kc%4)*128 :+128] of tile kc//4.  One DMA per
        # tile (4KB-contiguous DRAM rows; partition = h%128, c-dim = h//128).
        def load_w_half(base, khalf):
            t = wpool.tile([128, HC * 512], f16, tag=f"w{base}_{khalf}")
            nc.gpsimd.dma_start(
                t[:].rearrange("p (c k) -> p c k", k=512),
                w_attn[base:base + H, khalf * 512:(khalf + 1) * 512]
                .rearrange("(c p) k -> p c k", p=128))
            return t

        # w2a first (gates first GEMM), then the tiny operands (hidT/battn/
        # vT land instantly once issued), then w2b (GEMM kc4-7), w1a
        # (cbias kc0-3, bounds the tanh stall via psz recycling), nat01
        # (block-1 transposes), w1b last of the weights.
        w2h = [load_w_half(H, 0)]

        def w2s(kc, hc):
            return w2h[kc // 4][:, hc * 512 + (kc % 4) * 128:
                                hc * 512 + (kc % 4) * 128 + 128]

        hidT = cpool.tile([128, HC * B_LOC], f16, tag="hidT")
        for b in range(B_LOC):
            nc.gpsimd.dma_start(
                hidT[:].rearrange("p (c b) -> p c b", b=B_LOC)[:, :, b],
                hid[b].rearrange("(c p) -> p c", p=128))
        battnT = cpool.tile([128, KC], f32, tag="battnT")
        nc.gpsimd.dma_start(battnT[:], b_attn.ap().rearrange("(c p) -> p c", p=128))
        vT = cpool.tile([128, KC], f16, tag="vT")
        nc.gpsimd.dma_start(vT[:], v.ap().rearrange("(c p) -> p c", p=128))

        w2h.append(load_w_half(H, 1))
        w1h = [load_w_half(0, 0)]
        load_nat(0, 1)
        w1h.append(load_w_half(0, 1))

        def w1s(kc, hc):
            return w1h[kc // 4][:, hc * 512 + (kc % 4) * 128:
                                hc * 512 + (kc % 4) * 128 + 128]

        # remaining enc blocks stream behind the small operands
        for b in range(B_LOC):
            for rb in range(NRB):
                if b == 0 and rb <= 1:
                    continue
                load_nat(b, rb)

        # --- cbiasT[k, (kc, b)] = (hidden @ W1 + b_attn) transposed -------
        # W1 chunk is the stationary [h,k], hidT the moving [h,b]: the
        # matmul emits [k, b] directly, no transpose needed.
        cbiasT = cpool.tile([128, KC * B_LOC], f32, tag="cbiasT")
        for kc in range(KC):
            psh = pscbpool.tile([128, B_LOC], f32, tag="pscb")
            for hc in range(HC):
                nc.tensor.matmul(
                    psh[:], w1s(kc, hc),
                    hidT[:, hc * B_LOC:(hc + 1) * B_LOC],
                    start=(hc == 0), stop=(hc == HC - 1))
            nc.scalar.activation(
                cbiasT[:, kc * B_LOC:(kc + 1) * B_LOC], psh[:],
                AF.Identity, bias=battnT[:, kc:kc + 1])

        # --- main loop ----------------------------------------------------
        # Phase discipline: all PE transposes of block i+1 are ordered after
        # the last GEMM matmul of block i (same-engine, no semaphore), so the
        # PE alternates pure-transpose and pure-matmul phases.
        def do_transposes(b, rb, prev_anchor):
            encTs = []
            nt = nat[(b, rb)]
            for hc in range(HC):
                tt = tpool.tile([128, RB], f16, tag="encT")
                pt = pstpool.tile([128, RB], f16, tag="pst")
                for j in range(RB // 128):
                    tr = nc.tensor.transpose(
                        pt[:, j * 128:(j + 1) * 128],
                        nt[:, j * H + hc * 128: j * H + (hc + 1) * 128],
                        ident[:])
                    if prev_anchor is not None:
                        add_dep_helper(prev_anchor.ins, tr.ins,
                                       sync=False, reason="pe phase")
                nc.vector.tensor_copy(tt[:], pt[:])
                encTs.append(tt)
            return encTs

        blocks = [(b, rb) for b in range(B_LOC) for rb in range(NRB)]
        logits = {}
        for b in range(B_LOC):
            lg = smpool.tile([1, S], f32, tag=f"logits_{b}")
            logits[b] = lg

        # Per-block exp with accum runs hidden under the next block's GEMM;
        # only the final total/recip/scale/DMA are on the critical tail.
        expos = {}
        esums = {}
        for b in range(B_LOC):
            ex = smpool.tile([1, S], f32, tag=f"expo_{b}")
            es = smpool.tile([1, NRB], f32, tag=f"esum_{b}")
            expos[b] = ex
            esums[b] = es

        def block_exp(b, rb):
            # logits are O(1) so exp without max-subtraction is exact in math
            nc.scalar.activation(
                expos[b][:, rb * RB:(rb + 1) * RB],
                logits[b][:, rb * RB:(rb + 1) * RB], AF.Exp,
                accum_out=esums[b][:, rb:rb + 1])

        def epilogue(b):
            ssum = smpool.tile([1, 1], f32, tag=f"ssum_{b}")
            nc.vector.tensor_reduce(
                ssum[:], esums[b][:], axis=mybir.AxisListType.X, op=ALU.add)
            rec = smpool.tile([1, 1], f32, tag=f"rec_{b}")
            nc.vector.reciprocal(rec[:], ssum[:])
            # in-place scale keeps SBUF pressure down
            nc.scalar.activation(expos[b][:], expos[b][:], AF.Copy,
                                 scale=rec[:])
            nc.sync.dma_start(out[b:b + 1, :], expos[b][:])

        encTs_next = do_transposes(0, 0, None)
        pe_warm(16)  # first GEMM waits w2a (stall >= ~2.5us every run)
        for bi, (b, rb) in enumerate(blocks):
            if bi == 1:
                pe_warm(16)  # block-1 GEMM waits b0 tanh/cbias (w1-gated)
            encTs = encTs_next
            ens = []
            last_g = None
            for kc in range(KC):
                psz = pszpool.tile([128, RB], f32)
                for hc in range(HC):
                    last_g = nc.tensor.matmul(
                        psz[:], w2s(kc, hc), encTs[hc][:],
                        start=(hc == 0), stop=(hc == HC - 1))
                en = epool.tile([128, RB], f16, tag="energy")
                nc.scalar.activation(
                    en[:], psz[:], AF.Tanh,
                    bias=cbiasT[:, kc * B_LOC + b: kc * B_LOC + b + 1])
                ens.append(en)
            if bi + 1 < len(blocks):
                if bi == 0:
                    pe_warm(24)  # block-1 transposes wait nat01 (~5us)
                encTs_next = do_transposes(*blocks[bi + 1], last_g)
            psa = psattpool.tile([1, RB], f32)
            for kc in range(KC):
                nc.tensor.matmul(
                    psa[:], vT[:, kc:kc + 1], ens[kc][:],
                    start=(kc == 0), stop=(kc == KC - 1))
            nc.vector.tensor_copy(
                logits[b][:, rb * RB:(rb + 1) * RB], psa[:])
            block_exp(b, rb)
            if rb == NRB - 1:
                epilogue(b)

    nc.compile()
    return nc


_NC_CACHE = None


def _get_nc():
    global _NC_CACHE
    if _NC_CACHE is None:
        _NC_CACHE = build_kernel()
    return _NC_CACHE


def kernel(hidden, encoder_outputs, W_attn, b_attn, v, _trace=False,
           _tmpdir=None):
    hidden = np.ascontiguousarray(hidden, dtype=np.float32)
    encoder_outputs = np.ascontiguousarray(encoder_outputs, dtype=np.float32)
    W_attn = np.ascontiguousarray(W_attn, dtype=np.float32)
    b_attn = np.ascontiguousarray(b_attn, dtype=np.float32)
    v = np.ascontiguousarray(v, dtype=np.float32)

    nc = _get_nc()
    in_maps = []
    for c in range(N_CORES):
        b0 = c * B_LOC
        in_maps.append({
            "enc": encoder_outputs[b0:b0 + B_LOC],
            "hid": hidden[b0:b0 + B_LOC],
            "w_attn": W_attn,
            "b_attn": b_attn,
            "v": v,
        })
    res = run_bass_kernel_spmd(
        nc, in_maps, core_ids=list(range(N_CORES)),
        trace=_trace, tmpdir=_tmpdir)
    out = np.concatenate([res.results[c]["out"] for c in range(N_CORES)],
                         axis=0).astype(np.float32)
    if _trace:
        kernel.last_exec_time_ns = res.exec_time_ns
        kernel.last_results = res
    return out


# revision 28
# speedup vs baseline: 1.0377x; 1.0377x over previous
"""Trainium2 Bass kernel for additive (Bahdanau-style) attention scoring.

Computes, for hidden [B,H], encoder_outputs [B,S,H], W_attn [2H,H], b_attn [H], v [H]:
    energy    = tanh(hidden @ W1 + enc @ W2 + b_attn)   (per (b,s) row)
    attention = softmax_S(energy @ v)                   -> [B, S]

Sharding: data-parallel over batch across 8 NeuronCores (2 batches/core);
weights replicated.  Per-core compute is a 4096x1024x1024 GEMM + tanh +
v-dot + softmax, laid out as zT tiles [k=128 partitions, r free] so the
tanh bias is a per-partition AP on the scalar engine.  enc is cast
f32->fp16 during the HBM load (SWDGE) and transposed on-chip with PE
identity-matmul transposes whose PSUM results DVE-copies back to SBUF.

Improvements over the original baseline (224.5us -> ~194us on the same
device; the prologue is HBM-bound at ~285 GB/s effective, so DMA issue
order is everything):
- W1/W2 loaded with 2 big DMAs each instead of 8 small ones (each
  DMA_DIRECT2D costs ~1.2us of serial gpsimd queue time), ordered
  w2a -> tiny operands -> w2b -> w1a -> nat01 -> w1b so each consumer
  (first GEMM half, GEMM kc4-7, cbias via the psz-recycle slack,
  block-1 transposes) is fed just in time.
- nat pool bufs=8: every enc block's DMA issues up-front; no late
  mid-stream nat waits (was ~0.5us/block + 3.5us early).
- cbiasT computed directly as [k, b] (W1 chunk stationary, hidT
  moving), removing the extra PE transpose + second identity.
- Per-block exp+accum on the logits slice runs hidden under the next
  block's GEMM; the critical tail is only total/recip/scale/DMA.
- psz pool at 4 bufs (psep/psatt trimmed) for deeper GEMM/tanh overlap.

Measured dead ends (do not revisit): fp8 GEMM fails the 2e-2 gate
(8e-2); InstMatmult.ldweights=False is ignored by codegen (no
stationary reuse; ~46ns/matmul LDWEIGHTS exposure is a floor); DVE
tensor_scalar / scalar_tensor_tensor are ~10-40x slower than COPY;
GpSimd CROSS_LANE_REDUCE is ~77us per [128,512] tile; a [128,16]
partition-parallel softmax makes the output DMA a 4-byte scatter
(2048 descriptors, ~11.5us); HWDGE rings (sync/scalar only) do not
add bandwidth over the shared HBM limit.
"""

import sys
import types

import numpy as np

B, S, H = 16, 2048, 1024
N_CORES = 8
B_LOC = B // N_CORES  # 2 batches per core
HC = H // 128         # 8 contraction chunks
KC = H // 128         # 8 output-feature chunks
RB = 512              # rows (s positions) per block
NRB = S // RB         # 4 r-blocks per batch
NCH = S // 128        # 16 logit chunks of 128 per batch


def _ensure_axon_hooks():
    """Register the NTFF profile hook if the image's antenv lacks it."""
    try:
        import antenv.axon_hooks  # noqa: F401
        return
    except ImportError:
        pass
    try:
        import antenv
        from trn_agent_boot.trn_boot import _ntff_profile_via_ctypes
    except ImportError:
        return
    mod = types.ModuleType("antenv.axon_hooks")
    _hook = [None]
    mod.set_axon_ntff_profile_hook = lambda h: _hook.__setitem__(0, h)
    mod.get_axon_ntff_profile_hook = lambda: _hook[0]
    antenv.axon_hooks = mod
    sys.modules["antenv.axon_hooks"] = mod
    try:
        hook = _ntff_profile_via_ctypes("/opt/axon/libaxon_pjrt.so")
        mod.set_axon_ntff_profile_hook(hook)
    except Exception:
        pass


_ensure_axon_hooks()

import concourse.bass as bass  # noqa: E402,F401
import concourse.mybir as mybir  # noqa: E402
import concourse.tile as tile  # noqa: E402
from concourse import bacc  # noqa: E402
from concourse.bass_utils import run_bass_kernel_spmd  # noqa: E402
from concourse.masks import make_identity  # noqa: E402
from concourse.tile_rust import add_dep_helper  # noqa: E402

f32 = mybir.dt.float32
f16 = mybir.dt.float16
AF = mybir.ActivationFunctionType
ALU = mybir.AluOpType


def build_kernel():
    nc = bacc.Bacc("TRN2", target_bir_lowering=False, debug=False,
                   num_devices=N_CORES)

    enc = nc.dram_tensor("enc", [B_LOC, S, H], f32, kind="ExternalInput")
    hid = nc.dram_tensor("hid", [B_LOC, H], f32, kind="ExternalInput")
    w_attn = nc.dram_tensor("w_attn", [2 * H, H], f32, kind="ExternalInput")
    b_attn = nc.dram_tensor("b_attn", [H], f32, kind="ExternalInput")
    v = nc.dram_tensor("v", [H], f32, kind="ExternalInput")
    out = nc.dram_tensor("out", [B_LOC, S], f32, kind="ExternalOutput")

    with tile.TileContext(nc) as tc, \
         tc.tile_pool(name="weights", bufs=1) as wpool, \
         tc.tile_pool(name="consts", bufs=1) as cpool, \
         tc.tile_pool(name="nat", bufs=8) as natpool, \
         tc.tile_pool(name="encT", bufs=16) as tpool, \
         tc.tile_pool(name="energy", bufs=9) as epool, \
         tc.tile_pool(name="sm", bufs=1) as smpool, \
         tc.tile_pool(name="psz", bufs=4, space="PSUM") as pszpool, \
         tc.tile_pool(name="pst", bufs=2, space="PSUM") as pstpool, \
         tc.tile_pool(name="psatt", bufs=1, space="PSUM") as psattpool, \
         tc.tile_pool(name="pscb", bufs=1, space="PSUM") as pscbpool:

        # --- SWDGE FIFO ---------------------------------------------------
        # First enc block first (its transposes gate the whole pipeline),
        # identity for the PE transposes next, then big weight chunks.
        # Fewer + bigger DMAs: each DMA_DIRECT2D costs ~1.2us of gpsimd
        # queue time, and the whole prologue is HBM-bandwidth-bound
        # (~285 GB/s effective), so issue order = landing order.
        nat = {}

        def load_nat(b, rb, split=1):
            t = natpool.tile([128, (RB // 128) * H], f16, tag="nat")
            r0 = rb * RB
            step = (RB // 128) // split
            for s in range(split):
                nc.gpsimd.dma_start(
                    t[:].rearrange("p (j h) -> p j h", h=H)[
                        :, s * step:(s + 1) * step, :],
                    enc[b, r0 + s * step * 128:r0 + (s + 1) * step * 128, :]
                    .rearrange("(j p) h -> p j h", p=128))
            nat[(b, rb)] = t

        load_nat(0, 0, split=2)

        ident = cpool.tile([128, 128], f16, tag="ident")
        make_identity(nc, ident[:])


        # W2/W1 as two [128, (hc k)] half-k tiles each: slice for (kc, hc)
        # is [:, hc*512 + (kc%4)*128 :+128] of tile kc//4.  One DMA per
        # tile (4KB-contiguous DRAM rows; partition = h%128, c-dim = h//128).
        def load_w_half(base, khalf):
            t = wpool.tile([128, HC * 512], f16, tag=f"w{base}_{khalf}")
            nc.gpsimd.dma_start(
                t[:].rearrange("p (c k) -> p c k", k=512),
                w_attn[base:base + H, khalf * 512:(khalf + 1) * 512]
                .rearrange("(c p) k -> p c k", p=128))
            return t

        # w2a first (gates first GEMM), then the tiny operands (hidT/battn/
        # vT land instantly once issued), then w2b (GEMM kc4-7), w1a
        # (cbias kc0-3, bounds the tanh stall via psz recycling), nat01
        # (block-1 transposes), w1b last of the weights.
        w2h = [load_w_half(H, 0)]

        def w2s(kc, hc):
            return w2h[kc // 4][:, hc * 512 + (kc % 4) * 128:
                                hc * 512 + (kc % 4) * 128 + 128]

        hidT = cpool.tile([128, HC * B_LOC], f16, tag="hidT")
        for b in range(B_LOC):
            nc.gpsimd.dma_start(
                hidT[:].rearrange("p (c b) -> p c b", b=B_LOC)[:, :, b],
                hid[b].rearrange("(c p) -> p c", p=128))
        battnT = cpool.tile([128, KC], f32, tag="battnT")
        nc.gpsimd.dma_start(battnT[:], b_attn.ap().rearrange("(c p) -> p c", p=128))
        vT = cpool.tile([128, KC], f16, tag="vT")
        nc.gpsimd.dma_start(vT[:], v.ap().rearrange("(c p) -> p c", p=128))

        w2h.append(load_w_half(H, 1))
        w1h = [load_w_half(0, 0)]
        load_nat(0, 1)
        w1h.append(load_w_half(0, 1))

        def w1s(kc, hc):
            return w1h[kc // 4][:, hc * 512 + (kc % 4) * 128:
                                hc * 512 + (kc % 4) * 128 + 128]

        # remaining enc blocks stream behind the small operands
        for b in range(B_LOC):
            for rb in range(NRB):
                if b == 0 and rb <= 1:
                    continue
                load_nat(b, rb)

        # --- cbiasT[k, (kc, b)] = (hidden @ W1 + b_attn) transposed -------
        # W1 chunk is the stationary [h,k], hidT the moving [h,b]: the
        # matmul emits [k, b] directly, no transpose needed.
        cbiasT = cpool.tile([128, KC * B_LOC], f32, tag="cbiasT")
        for kc in range(KC):
            psh = pscbpool.tile([128, B_LOC], f32, tag="pscb")
            for hc in range(HC):
                nc.tensor.matmul(
                    psh[:], w1s(kc, hc),
                    hidT[:, hc * B_LOC:(hc + 1) * B_LOC],
                    start=(hc == 0), stop=(hc == HC - 1))
            nc.scalar.activation(
                cbiasT[:, kc * B_LOC:(kc + 1) * B_LOC], psh[:],
                AF.Identity, bias=battnT[:, kc:kc + 1])

        # --- main loop ----------------------------------------------------
        # Phase discipline: all PE transposes of block i+1 are ordered after
        # the last GEMM matmul of block i (same-engine, no semaphore), so the
        # PE alternates pure-transpose and pure-matmul phases.
        def do_transposes(b, rb, prev_anchor):
            encTs = []
            nt = nat[(b, rb)]
            for hc in range(HC):
                tt = tpool.tile([128, RB], f16, tag="encT")
                pt = pstpool.tile([128, RB], f16, tag="pst")
                for j in range(RB // 128):
                    tr = nc.tensor.transpose(
                        pt[:, j * 128:(j + 1) * 128],
                        nt[:, j * H + hc * 128: j * H + (hc + 1) * 128],
                        ident[:])
                    if prev_anchor is not None:
                        add_dep_helper(prev_anchor.ins, tr.ins,
                                       sync=False, reason="pe phase")
                nc.vector.tensor_copy(tt[:], pt[:])
                encTs.append(tt)
            return encTs

        blocks = [(b, rb) for b in range(B_LOC) for rb in range(NRB)]
        logits = {}
        for b in range(B_LOC):
            lg = smpool.tile([1, S], f32, tag=f"logits_{b}")
            logits[b] = lg

        # Per-block exp with accum runs hidden under the next block's GEMM;
        # only the final total/recip/scale/DMA are on the critical tail.
        expos = {}
        esums = {}
        for b in range(B_LOC):
            ex = smpool.tile([1, S], f32, tag=f"expo_{b}")
            es = smpool.tile([1, NRB], f32, tag=f"esum_{b}")
            expos[b] = ex
            esums[b] = es

        def block_exp(b, rb):
            # logits are O(1) so exp without max-subtraction is exact in math
            nc.scalar.activation(
                expos[b][:, rb * RB:(rb + 1) * RB],
                logits[b][:, rb * RB:(rb + 1) * RB], AF.Exp,
                accum_out=esums[b][:, rb:rb + 1])

        def epilogue(b):
            ssum = smpool.tile([1, 1], f32, tag=f"ssum_{b}")
            nc.vector.tensor_reduce(
                ssum[:], esums[b][:], axis=mybir.AxisListType.X, op=ALU.add)
            rec = smpool.tile([1, 1], f32, tag=f"rec_{b}")
            nc.vector.reciprocal(rec[:], ssum[:])
            # in-place scale keeps SBUF pressure down
            nc.scalar.activation(expos[b][:], expos[b][:], AF.Copy,
                                 scale=rec[:])
            nc.sync.dma_start(out[b:b + 1, :], expos[b][:])

        encTs_next = do_transposes(0, 0, None)
        for bi, (b, rb) in enumerate(blocks):
            encTs = encTs_next
            ens = []
            last_g = None
            for kc in range(KC):
                psz = pszpool.tile([128, RB], f32)
                for hc in range(HC):
                    last_g = nc.tensor.matmul(
                        psz[:], w2s(kc, hc), encTs[hc][:],
                        start=(hc == 0), stop=(hc == HC - 1))
                en = epool.tile([128, RB], f16, tag="energy")
                nc.scalar.activation(
                    en[:], psz[:], AF.Tanh,
                    bias=cbiasT[:, kc * B_LOC + b: kc * B_LOC + b + 1])
                ens.append(en)
            if bi + 1 < len(blocks):
                encTs_next = do_transposes(*blocks[bi + 1], last_g)
            psa = psattpool.tile([1, RB], f32)
            for kc in range(KC):
                nc.tensor.matmul(
                    psa[:], vT[:, kc:kc + 1], ens[kc][:],
                    start=(kc == 0), stop=(kc == KC - 1))
            nc.vector.tensor_copy(
                logits[b][:, rb * RB:(rb + 1) * RB], psa[:])
            block_exp(b, rb)
            if rb == NRB - 1:
                epilogue(b)

    nc.compile()
    return nc


_NC_CACHE = None


def _get_nc():
    global _NC_CACHE
    if _NC_CACHE is None:
        _NC_CACHE = build_kernel()
    return _NC_CACHE


def kernel(hidden, encoder_outputs, W_attn, b_attn, v, _trace=False,
           _tmpdir=None):
    hidden = np.ascontiguousarray(hidden, dtype=np.float32)
    encoder_outputs = np.ascontiguousarray(encoder_outputs, dtype=np.float32)
    W_attn = np.ascontiguousarray(W_attn, dtype=np.float32)
    b_attn = np.ascontiguousarray(b_attn, dtype=np.float32)
    v = np.ascontiguousarray(v, dtype=np.float32)

    nc = _get_nc()
    in_maps = []
    for c in range(N_CORES):
        b0 = c * B_LOC
        in_maps.append({
            "enc": encoder_outputs[b0:b0 + B_LOC],
            "hid": hidden[b0:b0 + B_LOC],
            "w_attn": W_attn,
            "b_attn": b_attn,
            "v": v,
        })
    res = run_bass_kernel_spmd(
        nc, in_maps, core_ids=list(range(N_CORES)),
        trace=_trace, tmpdir=_tmpdir)
    out = np.concatenate([res.results[c]["out"] for c in range(N_CORES)],
                         axis=0).astype(np.float32)
    if _trace:
        kernel.last_exec_time_ns = res.exec_time_ns
        kernel.last_results = res
    return out


# revision 29
# speedup vs baseline: 1.0602x; 1.0217x over previous
"""Trainium2 Bass kernel for additive (Bahdanau-style) attention scoring.

Computes, for hidden [B,H], encoder_outputs [B,S,H], W_attn [2H,H], b_attn [H], v [H]:
    energy    = tanh(hidden @ W1 + enc @ W2 + b_attn)   (per (b,s) row)
    attention = softmax_S(energy @ v)                   -> [B, S]

Sharding: data-parallel over batch across 8 NeuronCores (2 batches/core);
weights replicated.  Per-core compute is a 4096x1024x1024 GEMM + tanh +
v-dot + softmax, laid out as zT tiles [k=128 partitions, r free] so the
tanh bias is a per-partition AP on the scalar engine.  enc is cast
f32->fp16 during the HBM load (SWDGE) and transposed on-chip with PE
identity-matmul transposes whose PSUM results DVE-copies back to SBUF.

Improvements over the original baseline (224.5us -> ~194us on the same
device; the prologue is HBM-bound at ~285 GB/s effective, so DMA issue
order is everything):
- W1/W2 loaded with 2 big DMAs each instead of 8 small ones (each
  DMA_DIRECT2D costs ~1.2us of serial gpsimd queue time), ordered
  w2a -> tiny operands -> w2b -> w1a -> nat01 -> w1b so each consumer
  (first GEMM half, GEMM kc4-7, cbias via the psz-recycle slack,
  block-1 transposes) is fed just in time.
- nat pool bufs=8: every enc block's DMA issues up-front; no late
  mid-stream nat waits (was ~0.5us/block + 3.5us early).
- cbiasT computed directly as [k, b] (W1 chunk stationary, hidT
  moving), removing the extra PE transpose + second identity.
- Per-block exp+accum on the logits slice runs hidden under the next
  block's GEMM; the critical tail is only total/recip/scale/DMA.
- psz pool at 4 bufs (psep/psatt trimmed) for deeper GEMM/tanh overlap.

Measured dead ends (do not revisit): fp8 GEMM fails the 2e-2 gate
(8e-2); InstMatmult.ldweights=False is ignored by codegen (no
stationary reuse; ~46ns/matmul LDWEIGHTS exposure is a floor); DVE
tensor_scalar / scalar_tensor_tensor are ~10-40x slower than COPY;
GpSimd CROSS_LANE_REDUCE is ~77us per [128,512] tile; a [128,16]
partition-parallel softmax makes the output DMA a 4-byte scatter
(2048 descriptors, ~11.5us); HWDGE rings (sync/scalar only) do not
add bandwidth over the shared HBM limit.
"""

import sys
import types

import numpy as np

B, S, H = 16, 2048, 1024
N_CORES = 8
B_LOC = B // N_CORES  # 2 batches per core
HC = H // 128         # 8 contraction chunks
KC = H // 128         # 8 output-feature chunks
RB = 512              # rows (s positions) per block
NRB = S // RB         # 4 r-blocks per batch
NCH = S // 128        # 16 logit chunks of 128 per batch


def _ensure_axon_hooks():
    """Register the NTFF profile hook if the image's antenv lacks it."""
    try:
        import antenv.axon_hooks  # noqa: F401
        return
    except ImportError:
        pass
    try:
        import antenv
        from trn_agent_boot.trn_boot import _ntff_profile_via_ctypes
    except ImportError:
        return
    mod = types.ModuleType("antenv.axon_hooks")
    _hook = [None]
    mod.set_axon_ntff_profile_hook = lambda h: _hook.__setitem__(0, h)
    mod.get_axon_ntff_profile_hook = lambda: _hook[0]
    antenv.axon_hooks = mod
    sys.modules["antenv.axon_hooks"] = mod
    try:
        hook = _ntff_profile_via_ctypes("/opt/axon/libaxon_pjrt.so")
        mod.set_axon_ntff_profile_hook(hook)
    except Exception:
        pass


_ensure_axon_hooks()

import concourse.bass as bass  # noqa: E402,F401
import concourse.mybir as mybir  # noqa: E402
import concourse.tile as tile  # noqa: E402
from concourse import bacc  # noqa: E402
from concourse.bass_utils import run_bass_kernel_spmd  # noqa: E402
from concourse.masks import make_identity  # noqa: E402
from concourse.tile_rust import add_dep_helper  # noqa: E402

f32 = mybir.dt.float32
f16 = mybir.dt.float16
AF = mybir.ActivationFunctionType
ALU = mybir.AluOpType


def build_kernel():
    nc = bacc.Bacc("TRN2", target_bir_lowering=False, debug=False,
                   num_devices=N_CORES)

    enc = nc.dram_tensor("enc", [B_LOC, S, H], f32, kind="ExternalInput")
    hid = nc.dram_tensor("hid", [B_LOC, H], f32, kind="ExternalInput")
    w_attn = nc.dram_tensor("w_attn", [2 * H, H], f32, kind="ExternalInput")
    b_attn = nc.dram_tensor("b_attn", [H], f32, kind="ExternalInput")
    v = nc.dram_tensor("v", [H], f32, kind="ExternalInput")
    out = nc.dram_tensor("out", [B_LOC, S], f32, kind="ExternalOutput")

    with tile.TileContext(nc) as tc, \
         tc.tile_pool(name="weights", bufs=1) as wpool, \
         tc.tile_pool(name="consts", bufs=1) as cpool, \
         tc.tile_pool(name="nat", bufs=8) as natpool, \
         tc.tile_pool(name="encT", bufs=16) as tpool, \
         tc.tile_pool(name="energy", bufs=9) as epool, \
         tc.tile_pool(name="sm", bufs=1) as smpool, \
         tc.tile_pool(name="psz", bufs=4, space="PSUM") as pszpool, \
         tc.tile_pool(name="pst", bufs=2, space="PSUM") as pstpool, \
         tc.tile_pool(name="psatt", bufs=1, space="PSUM") as psattpool, \
         tc.tile_pool(name="pscb", bufs=1, space="PSUM") as pscbpool:

        # --- SWDGE FIFO ---------------------------------------------------
        # First enc block first (its transposes gate the whole pipeline),
        # identity for the PE transposes next, then big weight chunks.
        # Fewer + bigger DMAs: each DMA_DIRECT2D costs ~1.2us of gpsimd
        # queue time, and the whole prologue is HBM-bandwidth-bound
        # (~285 GB/s effective), so issue order = landing order.
        nat = {}

        def load_nat(b, rb, split=1):
            t = natpool.tile([128, (RB // 128) * H], f16, tag="nat")
            r0 = rb * RB
            step = (RB // 128) // split
            for s in range(split):
                nc.gpsimd.dma_start(
                    t[:].rearrange("p (j h) -> p j h", h=H)[
                        :, s * step:(s + 1) * step, :],
                    enc[b, r0 + s * step * 128:r0 + (s + 1) * step * 128, :]
                    .rearrange("(j p) h -> p j h", p=128))
            nat[(b, rb)] = t

        load_nat(0, 0, split=2)

        ident = cpool.tile([128, 128], f16, tag="ident")
        make_identity(nc, ident[:])


        # W2/W1 as two [128, (hc k)] half-k tiles each: slice for (kc, hc)
        # is [:, hc*512 + (kc%4)*128 :+128] of tile kc//4.  One DMA per
        # tile (4KB-contiguous DRAM rows; partition = h%128, c-dim = h//128).
        def load_w_half(base, khalf):
            t = wpool.tile([128, HC * 512], f16, tag=f"w{base}_{khalf}")
            nc.gpsimd.dma_start(
                t[:].rearrange("p (c k) -> p c k", k=512),
                w_attn[base:base + H, khalf * 512:(khalf + 1) * 512]
                .rearrange("(c p) k -> p c k", p=128))
            return t

        # w2a first (gates first GEMM), then the tiny operands (hidT/battn/
        # vT land instantly once issued), then w2b (GEMM kc4-7), w1a
        # (cbias kc0-3, bounds the tanh stall via psz recycling), nat01
        # (block-1 transposes), w1b last of the weights.
        w2h = [load_w_half(H, 0)]

        def w2s(kc, hc):
            return w2h[kc // 4][:, hc * 512 + (kc % 4) * 128:
                                hc * 512 + (kc % 4) * 128 + 128]

        hidT = cpool.tile([128, HC * B_LOC], f16, tag="hidT")
        for b in range(B_LOC):
            nc.gpsimd.dma_start(
                hidT[:].rearrange("p (c b) -> p c b", b=B_LOC)[:, :, b],
                hid[b].rearrange("(c p) -> p c", p=128))
        battnT = cpool.tile([128, KC], f32, tag="battnT")
        nc.gpsimd.dma_start(battnT[:], b_attn.ap().rearrange("(c p) -> p c", p=128))
        vT = cpool.tile([128, KC], f16, tag="vT")
        nc.gpsimd.dma_start(vT[:], v.ap().rearrange("(c p) -> p c", p=128))

        w2h.append(load_w_half(H, 1))
        w1h = [load_w_half(0, 0)]
        load_nat(0, 1)
        w1h.append(load_w_half(0, 1))

        def w1s(kc, hc):
            return w1h[kc // 4][:, hc * 512 + (kc % 4) * 128:
                                hc * 512 + (kc % 4) * 128 + 128]

        # remaining enc blocks stream behind the small operands
        for b in range(B_LOC):
            for rb in range(NRB):
                if b == 0 and rb <= 1:
                    continue
                load_nat(b, rb)

        # --- cbiasT[k, (kc, b)] = (hidden @ W1 + b_attn) transposed -------
        # W1 chunk is the stationary [h,k], hidT the moving [h,b]: the
        # matmul emits [k, b] directly, no transpose needed.
        cbiasT = cpool.tile([128, KC * B_LOC], f32, tag="cbiasT")
        for kc in range(KC):
            psh = pscbpool.tile([128, B_LOC], f32, tag="pscb")
            for hc in range(HC):
                nc.tensor.matmul(
                    psh[:], w1s(kc, hc),
                    hidT[:, hc * B_LOC:(hc + 1) * B_LOC],
                    start=(hc == 0), stop=(hc == HC - 1))
            nc.scalar.activation(
                cbiasT[:, kc * B_LOC:(kc + 1) * B_LOC], psh[:],
                AF.Identity, bias=battnT[:, kc:kc + 1])

        # --- main loop ----------------------------------------------------
        # Phase discipline: all PE transposes of block i+1 are ordered after
        # the last GEMM matmul of block i (same-engine, no semaphore), so the
        # PE alternates pure-transpose and pure-matmul phases.
        def do_transposes(b, rb, prev_anchor):
            encTs = []
            nt = nat[(b, rb)]
            for hc in range(HC):
                tt = tpool.tile([128, RB], f16, tag="encT")
                pt = pstpool.tile([128, RB], f16, tag="pst")
                for j in range(RB // 128):
                    tr = nc.tensor.transpose(
                        pt[:, j * 128:(j + 1) * 128],
                        nt[:, j * H + hc * 128: j * H + (hc + 1) * 128],
                        ident[:])
                    if prev_anchor is not None:
                        add_dep_helper(prev_anchor.ins, tr.ins,
                                       sync=False, reason="pe phase")
                nc.vector.tensor_copy(tt[:], pt[:])
                encTs.append(tt)
            return encTs

        blocks = [(b, rb) for b in range(B_LOC) for rb in range(NRB)]
        logits = {}
        for b in range(B_LOC):
            lg = smpool.tile([1, S], f32, tag=f"logits_{b}")
            logits[b] = lg

        # Per-block exp with accum runs hidden under the next block's GEMM;
        # only the final total/recip/scale/DMA are on the critical tail.
        expos = {}
        esums = {}
        for b in range(B_LOC):
            ex = smpool.tile([1, S], f32, tag=f"expo_{b}")
            es = smpool.tile([1, NRB], f32, tag=f"esum_{b}")
            expos[b] = ex
            esums[b] = es

        def block_exp(b, rb):
            # logits are O(1) so exp without max-subtraction is exact in math
            nc.scalar.activation(
                expos[b][:, rb * RB:(rb + 1) * RB],
                logits[b][:, rb * RB:(rb + 1) * RB], AF.Exp,
                accum_out=esums[b][:, rb:rb + 1])

        def epilogue(b):
            ssum = smpool.tile([1, 1], f32, tag=f"ssum_{b}")
            nc.vector.tensor_reduce(
                ssum[:], esums[b][:], axis=mybir.AxisListType.X, op=ALU.add)
            rec = smpool.tile([1, 1], f32, tag=f"rec_{b}")
            nc.vector.reciprocal(rec[:], ssum[:])
            # in-place scale keeps SBUF pressure down
            nc.scalar.activation(expos[b][:], expos[b][:], AF.Copy,
                                 scale=rec[:])
            nc.sync.dma_start(out[b:b + 1, :], expos[b][:])

        encTs_next = do_transposes(0, 0, None)
        for bi, (b, rb) in enumerate(blocks):
            encTs = encTs_next
            ens = []
            last_g = None
            for kc in range(KC):
                psz = pszpool.tile([128, RB], f32)
                for hc in range(HC):
                    last_g = nc.tensor.matmul(
                        psz[:], w2s(kc, hc), encTs[hc][:],
                        start=(hc == 0), stop=(hc == HC - 1))
                en = epool.tile([128, RB], f16, tag="energy")
                nc.scalar.activation(
                    en[:], psz[:], AF.Tanh,
                    bias=cbiasT[:, kc * B_LOC + b: kc * B_LOC + b + 1])
                ens.append(en)
            # v-dots first (matmul-mode, same stream as the GEMM), then
            # the next block's transposes: this keeps one contiguous
            # matmul-mode run per block and gives the nat(i+1) DMA an
            # extra ~1.7us before the transposes need it.
            psa = psattpool.tile([1, RB], f32)
            for kc in range(KC):
                last_g = nc.tensor.matmul(
                    psa[:], vT[:, kc:kc + 1], ens[kc][:],
                    start=(kc == 0), stop=(kc == KC - 1))
            if bi + 1 < len(blocks):
                encTs_next = do_transposes(*blocks[bi + 1], last_g)
            nc.vector.tensor_copy(
                logits[b][:, rb * RB:(rb + 1) * RB], psa[:])
            block_exp(b, rb)
            if rb == NRB - 1:
                epilogue(b)

    nc.compile()
    return nc


_NC_CACHE = None


def _get_nc():
    global _NC_CACHE
    if _NC_CACHE is None:
        _NC_CACHE = build_kernel()
    return _NC_CACHE


def kernel(hidden, encoder_outputs, W_attn, b_attn, v, _trace=False,
           _tmpdir=None):
    hidden = np.ascontiguousarray(hidden, dtype=np.float32)
    encoder_outputs = np.ascontiguousarray(encoder_outputs, dtype=np.float32)
    W_attn = np.ascontiguousarray(W_attn, dtype=np.float32)
    b_attn = np.ascontiguousarray(b_attn, dtype=np.float32)
    v = np.ascontiguousarray(v, dtype=np.float32)

    nc = _get_nc()
    in_maps = []
    for c in range(N_CORES):
        b0 = c * B_LOC
        in_maps.append({
            "enc": encoder_outputs[b0:b0 + B_LOC],
            "hid": hidden[b0:b0 + B_LOC],
            "w_attn": W_attn,
            "b_attn": b_attn,
            "v": v,
        })
    res = run_bass_kernel_spmd(
        nc, in_maps, core_ids=list(range(N_CORES)),
        trace=_trace, tmpdir=_tmpdir)
    out = np.concatenate([res.results[c]["out"] for c in range(N_CORES)],
                         axis=0).astype(np.float32)
    if _trace:
        kernel.last_exec_time_ns = res.exec_time_ns
        kernel.last_results = res
    return out
